# revision 16
# baseline (speedup 1.0000x reference)
"""Gaussian KDE (brute-force, bandwidth^2 = 1) on 8 Trainium2 NeuronCores.

Math:
    out_i = log( sum_j exp(x_i . y_j + b_j) ) - ||x_i||^2/2 - (d/2) log(2pi)
            - log(sum_j w_j),  with b_j = log(w_j) - ||y_j||^2/2.

Device work per core (queries sharded 8-way, 512 queries/core = 4 tiles of
128). The matmul is PRESCALED so PSUM holds u = (C1*s + C2 - 2^30)/256 as
fp32, where s = x.y + b, C1 = 2^23/ln2, C2 = K0 + SHIFT*C1:
    stationary rows = (C1/256)*x (bf16) + ones; moving rows = y (bf16) +
    hi/lo((C1/256)*b + (C2 - 2^30)/256).

Chunks of 1024 train-columns alternate between two independent consumers:
  'A' (ACT): table-exp with the free affine (scale=256/C1, bias=(2^30-K0)/C1
      recovers s+SHIFT exactly) and fused row-sum via accum_out. One pass.
  'D' (DVE): ONE custom fused op EXP_BITS_SUM_ANT:
      bits(out) = bits_f32(u + 1.5*2^23) << 8  == int32(C1*s + C2),
      i.e. the Schraudolph fast-exp bit pattern, with accum=ADD summing the
      bitcast floats in fp32.  (The fp add of the 1.5*2^23 magic rounds u to
      an integer held in the mantissa; the <<8 turns mantissa-integer into
      the full exponent+mantissa pattern; C2's 2^30 term restores the shifted
      out base.)

Final ln and per-query constants are applied on the host (4096 values), so
only the Exp table set is ever loaded on ACT.

Single-product bf16 scores (no hi/lo y split): max rel err ~1e-3 on the
output logs, far inside the 2e-2 gate. K = 34 rows (32 y + 2 bias).
"""

import numpy as np
import ml_dtypes

_Q, _N, _D = 4096, 65536, 32
_NCORES = 8
_QSHARD = _Q // _NCORES          # 512 queries per core
_QTILES = _QSHARD // 128         # 4 psum-partition tiles per core
_SUP = 2048                      # trains per supertile (one mv DMA tile)
_CHUNK = 1024                    # consumer chunk (2 PSUM banks)
_MMN = 512                       # moving free dim per matmul (1 psum bank)
_KROWS = 34                      # 32 y rows + 2 bias rows

_BF16 = ml_dtypes.bfloat16

# Uniform shift applied inside exp (folded back out on host): keeps the
# Schraudolph integers strictly positive for worst-case scores.
_SHIFT = 27.0
_C1 = float(2 ** 23 / np.log(2.0))
_C1S = _C1 / 256.0               # scale carried by the stationary x rows
_MAGIC = float(1.5 * 2 ** 23)    # fp-add magic: rounds to mantissa integer

# chunk consumer pattern, indexed by running chunk counter:
# 'A' = ACT table-exp w/ accum, 'D' = custom DVE Schraudolph w/ accum.
def _dqts(s):
    """Which qtiles of supertile s use the DVE path (rest use ACT)."""
    k = 2 if s % 3 != 2 else 1
    base = (2 * s) % 4
    return {(base + i) % 4 for i in range(k)}

_prog_cache: dict = {}
_exp_op_cache: list = []


def _k0_mean_zero():
    """127*2^23 - delta*2^23 with delta tuned so the relative error of the
    linear-mantissa exp approximation has zero mean over uniform fractions."""
    f = (np.arange(100000, dtype=np.float64) + 0.5) / 100000.0
    m0 = np.mean((1.0 + f) * 2.0 ** (-f))
    m1 = np.mean(2.0 ** (-f))
    delta = (m0 - 1.0) / m1
    return float(127 * 2 ** 23 - delta * 2 ** 23)


_K0 = _k0_mean_zero()
_C2 = _K0 + _SHIFT * _C1
_BOFF = (_C2 - 2.0 ** 30) / 256.0     # constant folded into the bias rows
_ACT_BIAS = (2.0 ** 30 - _K0) / _C1   # ACT affine bias: recovers s + SHIFT
_ACT_SCALE = 256.0 / _C1


def _build_program(n_trains: int):
    """Build the (identical-per-core) Bass program for n_trains train points."""
    import concourse.bass as bass
    import concourse.tile as tile
    from concourse import bacc, mybir

    f32 = mybir.dt.float32
    bf16 = mybir.dt.bfloat16
    i16 = mybir.dt.int16
    nsup = n_trains // _SUP

    nc = bacc.Bacc("TRN2", target_bir_lowering=False, debug=False,
                   num_devices=_NCORES)

    mv_d = nc.dram_tensor("mv", [_KROWS, n_trains], bf16, kind="ExternalInput")
    st_d = nc.dram_tensor("st", [_KROWS, _QSHARD], bf16, kind="ExternalInput")
    out_d = nc.dram_tensor("out", [128, _QTILES], f32, kind="ExternalOutput")

    ncol = (n_trains // _CHUNK)          # partial-sum cols per qtile

    with tile.TileContext(nc) as tc:
        with (
            tc.tile_pool(name="const", bufs=1) as cpool,
            tc.tile_pool(name="mv", bufs=3) as mvpool,
            tc.tile_pool(name="q", bufs=3) as qpool,
            tc.tile_pool(name="psum", bufs=4, space="PSUM") as ppool,
        ):
            st_sb = cpool.tile([64 + _KROWS, _QSHARD], bf16)
            nc.sync.dma_start(st_sb[0:_KROWS, :], st_d[:])
            nc.sync.dma_start(st_sb[64:64 + _KROWS, :], st_d[:])
            # per-(query-tile, chunk) partial sums
            sall = cpool.tile([128, _QTILES * ncol], f32)
            fin = cpool.tile([128, _QTILES], f32)
            bias_act = cpool.tile([128, 1], f32)
            nc.vector.memset(bias_act[:], _ACT_BIAS)
            scratch = cpool.tile([128, _SUP], bf16)
            nc.vector.memset(sall[:], 0.0)

            for s in range(nsup):
                mv_sb = mvpool.tile([64 + _KROWS, _SUP], bf16)
                nc.sync.dma_start(mv_sb[0:_KROWS, :],
                                  mv_d[:, s * _SUP:(s + 1) * _SUP])
                nc.sync.dma_start(mv_sb[64:64 + _KROWS, :],
                                  mv_d[:, s * _SUP:(s + 1) * _SUP])
                dq = _dqts(s)
                q16s = {}
                for qt in dq:
                    q16 = qpool.tile([128, _SUP], i16)
                    q16s[qt] = q16
                for h in range(_SUP // _CHUNK):
                    for qp in range(_QTILES // 2):
                        pss = []
                        for t in range(2):          # strip 0 / strip 64
                            qt = qp * 2 + t
                            base = 64 * t
                            ps = ppool.tile([128, _CHUNK], f32)
                            for j in range(_CHUNK // _MMN):
                                off = h * _CHUNK + j * _MMN
                                nc.tensor.matmul(
                                    out=ps[:, j * _MMN:(j + 1) * _MMN],
                                    lhsT=st_sb[base:base + _KROWS,
                                               qt * 128:(qt + 1) * 128],
                                    rhs=mv_sb[base:base + _KROWS,
                                              off:off + _MMN],
                                    start=True, stop=True,
                                )
                            pss.append((qt, ps))
                        for qt, ps in pss:
                            col = qt * ncol + s * (_SUP // _CHUNK) + h
                            if qt not in dq:
                                nc.scalar.activation(
                                    ps[:], ps[:],
                                    mybir.ActivationFunctionType.Exp,
                                    bias=bias_act[:], scale=_ACT_SCALE,
                                    accum_out=sall[:, col:col + 1],
                                )
                            else:
                                q16 = q16s[qt]
                                nc.vector.tensor_scalar(
                                    q16[:, h * _CHUNK:(h + 1) * _CHUNK],
                                    ps[:], 1.0 / 256.0, 16384.0,
                                    mybir.AluOpType.mult, mybir.AluOpType.add)
                for qt in dq:
                    col = qt * ncol + s * (_SUP // _CHUNK)
                    nc.vector.tensor_scalar(
                        scratch[:], q16s[qt][:].bitcast(bf16), 1.0,
                        None, mybir.AluOpType.mult, mybir.AluOpType.add,
                        accum_out=sall[:, col:col + 1])

            for qt in range(_QTILES):
                nc.vector.tensor_reduce(
                    fin[:, qt:qt + 1], sall[:, qt * ncol:(qt + 1) * ncol],
                    axis=mybir.AxisListType.X, op=mybir.AluOpType.add,
                )

            nc.sync.dma_start(out_d[:], fin[:])

    nc.compile()
    return nc


def _get_program(n_trains: int):
    if n_trains not in _prog_cache:
        _prog_cache[n_trains] = _build_program(n_trains)
    return _prog_cache[n_trains]


def _prep_inputs(X, X_train, sample_weight):
    X = np.ascontiguousarray(np.asarray(X, dtype=np.float32))
    Y = np.ascontiguousarray(np.asarray(X_train, dtype=np.float32))
    w = np.ascontiguousarray(np.asarray(sample_weight, dtype=np.float32))
    n = Y.shape[0]

    # per-train bias b_j = log w_j - ||y_j||^2/2, clipped so worst-case
    # Schraudolph integers stay positive; mapped to (C1/256)*b + BOFF and
    # hi/lo split in bf16 for ~2^-17 relative accuracy.
    w64 = w.astype(np.float64)
    b64 = np.log(np.maximum(w64, 1e-300)) - 0.5 * np.sum(
        Y.astype(np.float64) ** 2, axis=1)
    b64 = np.clip(b64, -35.0, None)
    bs = _C1S * b64 + _BOFF
    bhi = bs.astype(np.float32).astype(_BF16)
    blo = (bs - bhi.astype(np.float64)).astype(np.float32).astype(_BF16)

    mv = np.empty((_KROWS, n), dtype=_BF16)
    mv[0:32] = Y.T.astype(_BF16)
    mv[32] = bhi
    mv[33] = blo

    # per-query constant applied on host after the device sums
    const = 0.5 * _D * np.log(2.0 * np.pi) + np.log(np.sum(w64)) + _SHIFT
    dv_all = (0.5 * np.sum(X.astype(np.float64) ** 2, axis=1) + const)  # [Q]

    in_maps = []
    for c in range(_NCORES):
        xq = X[c * _QSHARD:(c + 1) * _QSHARD]          # [512, 32]
        st = np.empty((_KROWS, _QSHARD), dtype=_BF16)
        st[0:32] = (xq.T * _C1S).astype(_BF16)
        st[32:34] = np.ones((2, _QSHARD), dtype=_BF16)
        in_maps.append({"mv": mv, "st": st})
    return in_maps, dv_all


def _gather(results, dv_all):
    out = np.empty(_Q, dtype=np.float32)
    for c in range(_NCORES):
        res = results[c]["out"].astype(np.float64)      # [128, QTILES]
        sums = res.T.reshape(_QSHARD)                    # query-major
        lg = np.log(np.maximum(sums, 1e-300))
        out[c * _QSHARD:(c + 1) * _QSHARD] = (
            lg - dv_all[c * _QSHARD:(c + 1) * _QSHARD]).astype(np.float32)
    return out


def kernel(X, X_train, sample_weight, _want_timing=False):
    from concourse.bass_utils import run_bass_kernel_spmd

    nc = _get_program(_N)
    in_maps, dv_all = _prep_inputs(X, X_train, sample_weight)
    kres = run_bass_kernel_spmd(
        nc, in_maps, core_ids=list(range(_NCORES)),
        trace=bool(_want_timing),
    )
    out = _gather(kres.results, dv_all)
    if _want_timing:
        return out, kres
    return out


# revision 17
# speedup vs baseline: 1.1636x; 1.1636x over previous
"""Gaussian KDE (brute-force, bandwidth^2 = 1) on 8 Trainium2 NeuronCores.

Math:
    out_i = log( sum_j exp(x_i . y_j + b_j) ) - ||x_i||^2/2 - (d/2) log(2pi)
            - log(sum_j w_j),  with b_j = log(w_j) - ||y_j||^2/2.

Device work per core (queries sharded 8-way, 512 queries/core = 4 tiles of
128). The matmul is PRESCALED so PSUM holds u = (C1*s + C2 - 2^30)/256 as
fp32, where s = x.y + b, C1 = 2^23/ln2, C2 = K0 + SHIFT*C1:
    stationary rows = (C1/256)*x (bf16) + ones; moving rows = y (bf16) +
    hi/lo((C1/256)*b + (C2 - 2^30)/256).

Chunks of 1024 train-columns alternate between two independent consumers:
  'A' (ACT): table-exp with the free affine (scale=256/C1, bias=(2^30-K0)/C1
      recovers s+SHIFT exactly) and fused row-sum via accum_out. One pass.
  'D' (DVE): ONE custom fused op EXP_BITS_SUM_ANT:
      bits(out) = bits_f32(u + 1.5*2^23) << 8  == int32(C1*s + C2),
      i.e. the Schraudolph fast-exp bit pattern, with accum=ADD summing the
      bitcast floats in fp32.  (The fp add of the 1.5*2^23 magic rounds u to
      an integer held in the mantissa; the <<8 turns mantissa-integer into
      the full exponent+mantissa pattern; C2's 2^30 term restores the shifted
      out base.)

Final ln and per-query constants are applied on the host (4096 values), so
only the Exp table set is ever loaded on ACT.

Single-product bf16 scores (no hi/lo y split): max rel err ~1e-3 on the
output logs, far inside the 2e-2 gate. K = 34 rows (32 y + 2 bias).
"""

import numpy as np
import ml_dtypes

_Q, _N, _D = 4096, 65536, 32
_NCORES = 8
_QSHARD = _Q // _NCORES          # 512 queries per core
_QTILES = _QSHARD // 128         # 4 psum-partition tiles per core
_SUP = 2048                      # trains per supertile (one mv DMA tile)
_CHUNK = 1024                    # consumer chunk (2 PSUM banks)
_MMN = 512                       # moving free dim per matmul (1 psum bank)
_KROWS = 34                      # 32 y rows + 2 bias rows

_BF16 = ml_dtypes.bfloat16

# Uniform shift applied inside exp (folded back out on host): keeps the
# Schraudolph integers strictly positive for worst-case scores.
_SHIFT = 27.0
_C1 = float(2 ** 23 / np.log(2.0))
_C1S = _C1 / 256.0               # scale carried by the stationary x rows
_MAGIC = float(1.5 * 2 ** 23)    # fp-add magic: rounds to mantissa integer

# chunk consumer pattern, indexed by running chunk counter:
# 'A' = ACT table-exp w/ accum, 'D' = custom DVE Schraudolph w/ accum.
# per filled-pair consumer assignment: each entry maps (strip0, strip1)
# chunks to ACT ('A') or DVE ('D'); 9A:5D overall.
_PAIRPAT = ("AD", "DA", "AA", "AD", "DA", "AA", "AD")

_prog_cache: dict = {}
_exp_op_cache: list = []


def _k0_mean_zero():
    """127*2^23 - delta*2^23 with delta tuned so the relative error of the
    linear-mantissa exp approximation has zero mean over uniform fractions."""
    f = (np.arange(100000, dtype=np.float64) + 0.5) / 100000.0
    m0 = np.mean((1.0 + f) * 2.0 ** (-f))
    m1 = np.mean(2.0 ** (-f))
    delta = (m0 - 1.0) / m1
    return float(127 * 2 ** 23 - delta * 2 ** 23)


_K0 = _k0_mean_zero()
_C2 = _K0 + _SHIFT * _C1
_BOFF = (_C2 - 2.0 ** 30) / 256.0     # constant folded into the bias rows
_ACT_BIAS = (2.0 ** 30 - _K0) / _C1   # ACT affine bias: recovers s + SHIFT
_ACT_SCALE = 256.0 / _C1


def _build_program(n_trains: int):
    """Build the (identical-per-core) Bass program for n_trains train points."""
    import concourse.bass as bass
    import concourse.tile as tile
    from concourse import bacc, mybir

    f32 = mybir.dt.float32
    bf16 = mybir.dt.bfloat16
    i16 = mybir.dt.int16
    nsup = n_trains // _SUP

    nc = bacc.Bacc("TRN2", target_bir_lowering=False, debug=False,
                   num_devices=_NCORES)

    mv_d = nc.dram_tensor("mv", [_KROWS, n_trains], bf16, kind="ExternalInput")
    st_d = nc.dram_tensor("st", [_KROWS, _QSHARD], bf16, kind="ExternalInput")
    out_d = nc.dram_tensor("out", [128, _QTILES], f32, kind="ExternalOutput")

    ncol = (n_trains // _CHUNK)          # partial-sum cols per qtile

    with tile.TileContext(nc) as tc:
        with (
            tc.tile_pool(name="const", bufs=1) as cpool,
            tc.tile_pool(name="mv", bufs=3) as mvpool,
            tc.tile_pool(name="q", bufs=3) as qpool,
            tc.tile_pool(name="psum", bufs=4, space="PSUM") as ppool,
        ):
            st_sb = cpool.tile([64 + _KROWS, _QSHARD], bf16)
            nc.sync.dma_start(st_sb[0:_KROWS, :], st_d[:])
            nc.sync.dma_start(st_sb[64:64 + _KROWS, :], st_d[:])
            # per-(query-tile, chunk) partial sums
            sall = cpool.tile([128, _QTILES * ncol], f32)
            fin = cpool.tile([128, _QTILES], f32)
            bias_act = cpool.tile([128, 1], f32)
            nc.vector.memset(bias_act[:], _ACT_BIAS)
            scratch = cpool.tile([128, _SUP], bf16)
            nc.vector.memset(sall[:], 0.0)

            pair_ctr = 0
            for s in range(nsup):
                mv_sb = mvpool.tile([64 + _KROWS, _SUP], bf16)
                nc.sync.dma_start(mv_sb[0:_KROWS, :],
                                  mv_d[:, s * _SUP:(s + 1) * _SUP])
                nc.sync.dma_start(mv_sb[64:64 + _KROWS, :],
                                  mv_d[:, s * _SUP:(s + 1) * _SUP])
                for h in range(_SUP // _CHUNK):
                    for qp in range(_QTILES // 2):
                        kinds = _PAIRPAT[pair_ctr % len(_PAIRPAT)]
                        pair_ctr += 1
                        pss = []
                        for t in range(2):          # strip 0 / strip 64
                            qt = qp * 2 + t
                            base = 64 * t
                            ps = ppool.tile([128, _CHUNK], f32)
                            for j in range(_CHUNK // _MMN):
                                off = h * _CHUNK + j * _MMN
                                nc.tensor.matmul(
                                    out=ps[:, j * _MMN:(j + 1) * _MMN],
                                    lhsT=st_sb[base:base + _KROWS,
                                               qt * 128:(qt + 1) * 128],
                                    rhs=mv_sb[base:base + _KROWS,
                                              off:off + _MMN],
                                    start=True, stop=True,
                                )
                            pss.append((qt, ps, kinds[t]))
                        for qt, ps, kind in pss:
                            col = qt * ncol + s * (_SUP // _CHUNK) + h
                            if kind == "A":
                                nc.scalar.activation(
                                    ps[:], ps[:],
                                    mybir.ActivationFunctionType.Exp,
                                    bias=bias_act[:], scale=_ACT_SCALE,
                                    accum_out=sall[:, col:col + 1],
                                )
                            else:
                                q16 = qpool.tile([128, _CHUNK], i16)
                                nc.vector.tensor_scalar(
                                    q16[:], ps[:], 1.0 / 256.0, 16384.0,
                                    mybir.AluOpType.mult, mybir.AluOpType.add)
                                nc.vector.tensor_scalar(
                                    scratch[:, 0:_CHUNK], q16[:].bitcast(bf16),
                                    1.0, None, mybir.AluOpType.mult,
                                    mybir.AluOpType.add,
                                    accum_out=sall[:, col:col + 1])

            for qt in range(_QTILES):
                nc.vector.tensor_reduce(
                    fin[:, qt:qt + 1], sall[:, qt * ncol:(qt + 1) * ncol],
                    axis=mybir.AxisListType.X, op=mybir.AluOpType.add,
                )

            nc.sync.dma_start(out_d[:], fin[:])

    nc.compile()
    return nc


def _get_program(n_trains: int):
    if n_trains not in _prog_cache:
        _prog_cache[n_trains] = _build_program(n_trains)
    return _prog_cache[n_trains]


def _prep_inputs(X, X_train, sample_weight):
    X = np.ascontiguousarray(np.asarray(X, dtype=np.float32))
    Y = np.ascontiguousarray(np.asarray(X_train, dtype=np.float32))
    w = np.ascontiguousarray(np.asarray(sample_weight, dtype=np.float32))
    n = Y.shape[0]

    # per-train bias b_j = log w_j - ||y_j||^2/2, clipped so worst-case
    # Schraudolph integers stay positive; mapped to (C1/256)*b + BOFF and
    # hi/lo split in bf16 for ~2^-17 relative accuracy.
    w64 = w.astype(np.float64)
    b64 = np.log(np.maximum(w64, 1e-300)) - 0.5 * np.sum(
        Y.astype(np.float64) ** 2, axis=1)
    b64 = np.clip(b64, -35.0, None)
    bs = _C1S * b64 + _BOFF
    bhi = bs.astype(np.float32).astype(_BF16)
    blo = (bs - bhi.astype(np.float64)).astype(np.float32).astype(_BF16)

    mv = np.empty((_KROWS, n), dtype=_BF16)
    mv[0:32] = Y.T.astype(_BF16)
    mv[32] = bhi
    mv[33] = blo

    # per-query constant applied on host after the device sums
    const = 0.5 * _D * np.log(2.0 * np.pi) + np.log(np.sum(w64)) + _SHIFT
    dv_all = (0.5 * np.sum(X.astype(np.float64) ** 2, axis=1) + const)  # [Q]

    in_maps = []
    for c in range(_NCORES):
        xq = X[c * _QSHARD:(c + 1) * _QSHARD]          # [512, 32]
        st = np.empty((_KROWS, _QSHARD), dtype=_BF16)
        st[0:32] = (xq.T * _C1S).astype(_BF16)
        st[32:34] = np.ones((2, _QSHARD), dtype=_BF16)
        in_maps.append({"mv": mv, "st": st})
    return in_maps, dv_all


def _gather(results, dv_all):
    out = np.empty(_Q, dtype=np.float32)
    for c in range(_NCORES):
        res = results[c]["out"].astype(np.float64)      # [128, QTILES]
        sums = res.T.reshape(_QSHARD)                    # query-major
        lg = np.log(np.maximum(sums, 1e-300))
        out[c * _QSHARD:(c + 1) * _QSHARD] = (
            lg - dv_all[c * _QSHARD:(c + 1) * _QSHARD]).astype(np.float32)
    return out


def kernel(X, X_train, sample_weight, _want_timing=False):
    from concourse.bass_utils import run_bass_kernel_spmd

    nc = _get_program(_N)
    in_maps, dv_all = _prep_inputs(X, X_train, sample_weight)
    kres = run_bass_kernel_spmd(
        nc, in_maps, core_ids=list(range(_NCORES)),
        trace=bool(_want_timing),
    )
    out = _gather(kres.results, dv_all)
    if _want_timing:
        return out, kres
    return out
